# revision 7
# baseline (speedup 1.0000x reference)
"""GCN 3-layer Bass kernel for nn_ActionNetwork_20401094656134 on 8 trn2 cores.

Plan (node-parallel, dst-sharded):
- augment edges (drop self loops, add one per node); deg = in-degree
- nodes snake-dealt by degree to 8 shards; within shard, nodes get a parity
  bit (greedy discrepancy balancing of each dst's source parities) and are
  placed at matching-parity table positions sorted by (b,a) class counts
- scaled-table formulation: maintaining h~ = rsqrt(deg) * h makes each layer
      S[d] = sum_{e: dst=d} h~[src_e] + h~[d]   (self loop is a real slot)
      z    = (dis * S) @ W + b,   next h~ = dis * relu(z)
- per layer: every core holds the full bf16 h~ table in HBM (AllGather),
  gathers its ELL slots with batched SWDGE dma_gather (128B rows addressed
  as halves of 256B pairs; int16 idx = pair - 32768; even/odd source-row
  classes), segment-sums via identity-weight PE matmuls accumulating in
  PSUM, and runs the tiny per-tile tail (scale, transpose, W matmul, bias,
  relu-scale) on DVE/PE/ACT.
"""
import sys
sys.path.insert(0, "/opt/trn_rl_repo")

import numpy as np
import ml_dtypes

N_NODES = 100000
D = 64
OUT = 4
C = 8
P = 128
TILES = 98
SHARD_PAD = TILES * P          # 12544
NPS = N_NODES // C             # 12500
N_DUMMY = SHARD_PAD - NPS      # 44
IDX_BASE = 32768               # pair-unit base for int16 idx
PAD_PAIR = (6 * SHARD_PAD) // 2  # shard-6 dummy rows 75264/75265
PAD_IDX = PAD_PAIR - IDX_BASE
TABLE_ROWS = C * SHARD_PAD     # 100352
GK = 120                       # max data columns per gather group


# ---------------------------------------------------------------- host prep
def _build_structure(edge_index):
    n = N_NODES
    src = np.asarray(edge_index[0], np.int64)
    dst = np.asarray(edge_index[1], np.int64)
    keep = src != dst
    src, dst = src[keep], dst[keep]
    src = np.concatenate([src, np.arange(n, dtype=np.int64)])
    dst = np.concatenate([dst, np.arange(n, dtype=np.int64)])
    deg = np.bincount(dst, minlength=n).astype(np.int64)

    order = np.argsort(deg, kind="stable")
    shard_of = np.empty(n, np.int64)
    for i in range(0, n, 2 * C):
        blk = order[i:i + 2 * C]
        shard_of[blk[:C]] = np.arange(len(blk[:C]))
        bwd = blk[C:]
        shard_of[bwd] = np.arange(C - 1, C - 1 - len(bwd), -1)

    ord_e = np.argsort(dst, kind="stable")
    src_s = src[ord_e]
    starts = np.zeros(n + 1, np.int64)
    np.cumsum(deg, out=starts[1:])

    # greedy parity assignment balancing each dst's source parity split
    ord_s = np.argsort(src, kind="stable")
    dst_bysrc = dst[ord_s]
    odeg = np.bincount(src, minlength=n).astype(np.int64)
    sstarts = np.zeros(n + 1, np.int64)
    np.cumsum(odeg, out=sstarts[1:])

    parity = np.zeros(n, np.int8)
    imbal = np.zeros(n, np.int32)
    quota = np.zeros((C, 2), np.int64)
    quota[:, 0] = NPS // 2
    quota[:, 1] = NPS - NPS // 2
    for nd in order[::-1]:
        c = shard_of[nd]
        lo, hi = sstarts[nd], sstarts[nd + 1]
        vote = int(imbal[dst_bysrc[lo:hi]].sum())
        if quota[c, 0] == 0:
            p = 1
        elif quota[c, 1] == 0:
            p = 0
        else:
            p = 0 if vote <= 0 else 1
        parity[nd] = p
        quota[c, p] -= 1
        imbal[dst_bysrc[lo:hi]] += 1 - 2 * p

    # class counts per node
    b_cnt = np.zeros(n, np.int64)
    np.add.at(b_cnt, dst, parity[src])
    a_cnt = deg - b_cnt

    # positions: per shard, parity streams sorted by (b, a), dummies first
    pos_of = np.empty(n, np.int64)
    for c in range(C):
        nodes_c = np.where(shard_of == c)[0]
        pc = parity[nodes_c]
        ev = nodes_c[pc == 0]
        od = nodes_c[pc == 1]
        ev = ev[np.lexsort((a_cnt[ev], b_cnt[ev]))]
        od = od[np.lexsort((a_cnt[od], b_cnt[od]))]
        pos_of[ev] = 2 * (SHARD_PAD // 2 - len(ev)) + 2 * np.arange(len(ev))
        pos_of[od] = 2 * (SHARD_PAD // 2 - len(od)) + 2 * np.arange(len(od)) + 1

    table_row = shard_of * SHARD_PAD + pos_of
    src_rows = table_row[src_s]
    par = src_rows & 1
    b_cnt = np.zeros(n, np.int64)
    np.add.at(b_cnt, dst[ord_e], par)
    a_cnt = deg - b_cnt

    node_at = np.full((C, SHARD_PAD), -1, np.int64)
    for c in range(C):
        nodes_c = np.where(shard_of == c)[0]
        node_at[c, pos_of[nodes_c]] = nodes_c

    a_pad = np.zeros((C, SHARD_PAD), np.int64)
    b_pad = np.zeros((C, SHARD_PAD), np.int64)
    m = node_at >= 0
    a_pad[m] = a_cnt[node_at[m]]
    b_pad[m] = b_cnt[node_at[m]]
    A_t = np.maximum(a_pad.reshape(C, TILES, P).max(axis=(0, 2)), 1)
    B_t = np.maximum(b_pad.reshape(C, TILES, P).max(axis=(0, 2)), 1)

    # groups of tiles
    groups = []
    cur, cur_cols = [], 0
    for t in range(TILES):
        cols = int(A_t[t] + B_t[t])
        if cur and cur_cols + cols > GK:
            groups.append(cur)
            cur, cur_cols = [], 0
        cur.append(t)
        cur_cols += cols
    if cur:
        groups.append(cur)

    ge_cols = [int(sum(A_t[t] for t in g)) + 1 for g in groups]  # +1 pad col
    go_cols = [int(sum(B_t[t] for t in g)) + 1 for g in groups]
    tot_cols = sum(ge_cols) + sum(go_cols)

    pair_idx = (src_rows >> 1) - IDX_BASE

    # per-core column-major slot grid [tot_cols, P], then wrap to int16 SBUF
    idx_wrapped = np.empty((C, P, 8 * tot_cols), np.int16)
    for c in range(C):
        cols_all = np.full((tot_cols, P), PAD_IDX, np.int64)
        col0 = 0
        for gi, g in enumerate(groups):
            for t in g:
                for p in range(P):
                    nd = node_at[c, t * P + p]
                    if nd < 0:
                        continue
                    s0, s1 = starts[nd], starts[nd + 1]
                    pe = pair_idx[s0:s1][par[s0:s1] == 0]
                    cols_all[col0:col0 + len(pe), p] = pe
                col0 += int(A_t[t])
            col0 += 1
            for t in g:
                for p in range(P):
                    nd = node_at[c, t * P + p]
                    if nd < 0:
                        continue
                    s0, s1 = starts[nd], starts[nd + 1]
                    po = pair_idx[s0:s1][par[s0:s1] == 1]
                    cols_all[col0:col0 + len(po), p] = po
                col0 += int(B_t[t])
            col0 += 1
        assert col0 == tot_cols
        L = cols_all.reshape(-1).astype(np.int16)
        Mw = L.reshape(-1, 16).T
        idx_wrapped[c] = np.tile(Mw, (8, 1))

    return dict(
        deg=deg, node_at=node_at, groups=groups, A_t=A_t, B_t=B_t,
        ge_cols=ge_cols, go_cols=go_cols, tot_cols=tot_cols,
        idx_wrapped=idx_wrapped,
    )


# ------------------------------------------------- patched batched dma_gather
def _dma_gather_128(gpsimd, mybir, ap_utils, out_ap, in_ap, idxs_ap,
                    num_idxs, elem_size, elem_step, queue_num):
    """dma_gather with 128-byte elements (relaxes bass's %256 assert; the Q7
    non-transpose path has no such requirement — only the 256B *stride*
    granularity is ISA-level, which the pair-view satisfies)."""
    from concourse.bass import MemorySpace, exact_div, round_up_to_multiple
    self = gpsimd
    assert idxs_ap.dtype == mybir.dt.int16
    assert in_ap.dtype == out_ap.dtype
    elem_size_bytes = elem_size * mybir.dt.size(in_ap.dtype)
    assert elem_size_bytes % 128 == 0
    assert in_ap.space == MemorySpace.DRAM
    assert ap_utils.ap_is_contiguous(out_ap.ap[1:])
    assert ap_utils.ap_is_contiguous(idxs_ap.ap[1:])
    assert num_idxs % 128 == 0
    assert in_ap.ap[-1][1] == out_ap.ap[-1][1] == elem_size
    assert out_ap.ap[0][1] * out_ap.ap[1][1] == round_up_to_multiple(num_idxs, 128)
    assert in_ap.ap[0][0] == elem_step
    stride_bytes = elem_step * mybir.dt.size(in_ap.dtype)
    stride_bytes_256 = exact_div(stride_bytes, 256)
    assert stride_bytes_256 < 256
    self._assert_queue_num(queue_num)

    _in_ap = self.lower_ap_dma(in_ap, for_custom_bir_dma=True)
    _idxs_ap = self.lower_ap(idxs_ap)
    _out_ap = self.lower_ap(out_ap)
    return self.add_instruction(
        mybir.InstDMAGatherAnt(
            name=self.bass.get_next_instruction_name(),
            ins=[*_in_ap, _idxs_ap,
                 self.lower_val_access(self.to_reg(num_idxs))],
            outs=[_out_ap],
            transpose=False,
            num_idxs=num_idxs,
            elem_size=elem_size,
            stride_bytes_256=stride_bytes_256,
            gen_mode=0,
            single_packet=False,
            queue_num=queue_num,
            sbuf_tokens_per_rank=0,
            sbuf_free_dim_per_rank=0,
            sbuf_free_dim_pad_per_rank=0,
            sbuf_byte_offset=0,
        )
    )


# ------------------------------------------------------------ device program
def _build_program(S):
    import concourse.bass as bass
    import concourse.bacc as bacc
    import concourse.tile as tile
    import concourse.mybir as mybir
    import concourse.ap_utils as ap_utils
    from concourse.masks import make_identity

    bf16 = mybir.dt.bfloat16
    f32 = mybir.dt.float32
    groups, A_t, B_t = S["groups"], S["A_t"], S["B_t"]
    ge_cols, go_cols, tot_cols = S["ge_cols"], S["go_cols"], S["tot_cols"]
    max_gcols = max(ge + go for ge, go in zip(ge_cols, go_cols))

    nc = bacc.Bacc("TRN2", target_bir_lowering=False, debug=False,
                   num_devices=C, num_swdge_queues=4,
                   dynamic_dma_scratch_size=65536)

    in_x = nc.dram_tensor("x_shard", (SHARD_PAD, D), f32, kind="ExternalInput").ap()
    in_deg = nc.dram_tensor("degs", (P, TILES), f32, kind="ExternalInput").ap()
    in_idx = nc.dram_tensor("idxw", (P, 8 * tot_cols), mybir.dt.int16,
                            kind="ExternalInput").ap()
    in_W0 = nc.dram_tensor("W0", (D, D), bf16, kind="ExternalInput").ap()
    in_W1 = nc.dram_tensor("W1", (D, D), bf16, kind="ExternalInput").ap()
    in_W2 = nc.dram_tensor("W2", (D, OUT), bf16, kind="ExternalInput").ap()
    in_b0 = nc.dram_tensor("b0", (P, D), f32, kind="ExternalInput").ap()
    in_b1 = nc.dram_tensor("b1", (P, D), f32, kind="ExternalInput").ap()
    in_b2 = nc.dram_tensor("b2", (P, OUT), f32, kind="ExternalInput").ap()
    out_t = nc.dram_tensor("out", (SHARD_PAD, OUT), f32, kind="ExternalOutput").ap()

    with tile.TileContext(nc) as tc:
        with tc.tile_pool(name="const", bufs=1) as constp, \
             tc.tile_pool(name="dram", bufs=1, space="DRAM") as dram, \
             tc.tile_pool(name="xin", bufs=3) as xin, \
             tc.tile_pool(name="stgp", bufs=2) as stgp, \
             tc.tile_pool(name="work", bufs=3) as work, \
             tc.tile_pool(name="accp", bufs=3, space="PSUM") as accp, \
             tc.tile_pool(name="tpsp", bufs=2, space="PSUM") as tpsp, \
             tc.tile_pool(name="zpsp", bufs=2, space="PSUM") as zpsp:

            ident = constp.tile([P, P], bf16)
            make_identity(nc, ident[:])
            W0_sb = constp.tile([D, D], bf16, tag="W0")
            W1_sb = constp.tile([D, D], bf16, tag="W1")
            W2_sb = constp.tile([D, OUT], bf16, tag="W2")
            W_sb = [W0_sb, W1_sb, W2_sb]
            nc.sync.dma_start(W_sb[0][:], in_W0[:])
            nc.sync.dma_start(W_sb[1][:], in_W1[:])
            nc.sync.dma_start(W_sb[2][:], in_W2[:])
            b0_sb = constp.tile([P, D], f32, tag="b0")
            b1_sb = constp.tile([P, D], f32, tag="b1")
            b2_sb = constp.tile([P, OUT], f32, tag="b2")
            b_sb = [b0_sb, b1_sb, b2_sb]
            nc.sync.dma_start(b_sb[0][:], in_b0[:])
            nc.sync.dma_start(b_sb[1][:], in_b1[:])
            nc.sync.dma_start(b_sb[2][:], in_b2[:])

            idx_sb = constp.tile([P, 8 * tot_cols], mybir.dt.int16)
            nc.sync.dma_start(idx_sb[:], in_idx[:])

            deg_sb = constp.tile([P, TILES], f32)
            nc.sync.dma_start(deg_sb[:], in_deg[:])
            rcp = constp.tile([P, TILES], f32)
            nc.vector.reciprocal(rcp[:], deg_sb[:])
            dis = constp.tile([P, TILES], f32)
            nc.scalar.sqrt(dis[:], rcp[:])

            zpad = constp.tile([P, D], bf16)
            nc.gpsimd.memset(zpad[:], 0.0)

            tblA = dram.tile([TABLE_ROWS, D], bf16)
            tblB = dram.tile([TABLE_ROWS, D], bf16)
            agin = dram.tile([SHARD_PAD, D], bf16)

            # ---- layer-0 table: h0~ = dis * x ----
            for t in range(TILES):
                xt = xin.tile([P, D], f32, tag="xt")
                nc.sync.dma_start(xt[:], in_x[t * P:(t + 1) * P, :])
                h0 = xin.tile([P, D], bf16, tag="h0")
                nc.vector.tensor_scalar_mul(h0[:], xt[:], dis[:, t:t + 1])
                nc.sync.dma_start(agin[t * P:(t + 1) * P, :], h0[:])
            nc.gpsimd.collective_compute(
                "AllGather", mybir.AluOpType.bypass,
                replica_groups=[list(range(C))],
                ins=[agin[:].opt()], outs=[tblA[:].opt()],
            )

            qrr = 0
            for layer in range(3):
                table = tblA if layer % 2 == 0 else tblB
                tblP = table[:].rearrange("(r two) d -> r (two d)", two=2)
                Wl = W_sb[layer]
                bl = b_sb[layer]
                DO = D if layer < 2 else OUT
                wcol0 = 0
                for gi, g in enumerate(groups):
                    Acols, Bcols = ge_cols[gi], go_cols[gi]
                    cols_g = Acols + Bcols
                    stg = stgp.tile([P, max_gcols * D], bf16, tag="stg")
                    stg3 = stg[:].rearrange("p (k d) -> p k d", k=max_gcols)
                    _dma_gather_128(
                        nc.gpsimd, mybir, ap_utils,
                        stg3[:, 0:Acols, :], tblP[IDX_BASE:, 0:D],
                        idx_sb[:, 8 * wcol0:8 * (wcol0 + Acols)],
                        num_idxs=P * Acols, elem_size=D, elem_step=2 * D,
                        queue_num=qrr % 4,
                    )
                    _dma_gather_128(
                        nc.gpsimd, mybir, ap_utils,
                        stg3[:, Acols:cols_g, :], tblP[IDX_BASE:, D:2 * D],
                        idx_sb[:, 8 * (wcol0 + Acols):8 * (wcol0 + cols_g)],
                        num_idxs=P * Bcols, elem_size=D, elem_step=2 * D,
                        queue_num=(qrr + 1) % 4,
                    )
                    qrr += 2
                    wcol0 += cols_g

                    aoff = 0
                    boff = Acols
                    for t in g:
                        ka, kb = int(A_t[t]), int(B_t[t])
                        acc = accp.tile([P, D], f32, space="PSUM", tag="acc")
                        slot_cols = ([aoff + k for k in range(ka)] +
                                     [boff + k for k in range(kb)])
                        for j, col in enumerate(slot_cols):
                            nc.tensor.matmul(
                                out=acc[:], lhsT=ident[:],
                                rhs=stg[:, col * D:(col + 1) * D],
                                start=(j == 0), stop=(j == len(slot_cols) - 1),
                            )
                        aoff += ka
                        boff += kb

                        ssc = work.tile([P, D], bf16, tag="ssc")
                        nc.vector.tensor_scalar_mul(ssc[:], acc[:], dis[:, t:t + 1])
                        tps = tpsp.tile([D, P], bf16, space="PSUM", tag="tps")
                        nc.tensor.transpose(out=tps[:], in_=ssc[:], identity=ident[:])
                        stt = work.tile([D, P], bf16, tag="stt")
                        nc.vector.tensor_copy(stt[:], tps[:])
                        zps = zpsp.tile([P, DO], f32, space="PSUM", tag="zps")
                        nc.tensor.matmul(out=zps[:], lhsT=stt[:], rhs=Wl[:],
                                         start=True, stop=True)
                        if layer < 2:
                            zf = work.tile([P, D], f32, tag="zf")
                            nc.vector.tensor_tensor(out=zf[:], in0=zps[:],
                                                    in1=bl[:],
                                                    op=mybir.AluOpType.add)
                            hb = work.tile([P, D], bf16, tag="hb")
                            nc.scalar.activation(
                                hb[:], zf[:], mybir.ActivationFunctionType.Relu,
                                scale=dis[:, t:t + 1])
                            nc.sync.dma_start(agin[t * P:(t + 1) * P, :], hb[:])
                        else:
                            zf = work.tile([P, OUT], f32, tag="zfo")
                            nc.vector.tensor_tensor(out=zf[:], in0=zps[:],
                                                    in1=bl[:],
                                                    op=mybir.AluOpType.add)
                            nc.sync.dma_start(out_t[t * P:(t + 1) * P, :], zf[:])

                if layer < 2:
                    # dummy rows must stay exactly zero in the table
                    nc.sync.dma_start(agin[0:N_DUMMY, :], zpad[0:N_DUMMY, :])
                    nxt = tblB if layer % 2 == 0 else tblA
                    nc.gpsimd.collective_compute(
                        "AllGather", mybir.AluOpType.bypass,
                        replica_groups=[list(range(C))],
                        ins=[agin[:].opt()], outs=[nxt[:].opt()],
                    )

    nc.compile()
    return nc


def _bench_exec(nc, in_maps, iters):
    """Steady-state timing of the NEFF via repeated PJRT executions (no
    donation, device-resident inputs). Returns min per-iteration ns."""
    import time
    import jax
    import numpy as np
    from jax.sharding import Mesh, PartitionSpec
    from jax.experimental.shard_map import shard_map
    import concourse.mybir as mybir
    from concourse import bass2jax

    bass2jax.install_neuronx_cc_hook()
    partition_name = (nc.partition_id_tensor.name
                      if nc.partition_id_tensor else None)
    in_names, out_names, out_avals, zero_outs = [], [], [], []
    for alloc in nc.m.functions[0].allocations:
        if not isinstance(alloc, mybir.MemoryLocationSet):
            continue
        name = alloc.memorylocations[0].name
        if alloc.kind == "ExternalInput":
            if name != partition_name:
                in_names.append(name)
        elif alloc.kind == "ExternalOutput":
            out_names.append(name)
            shape = tuple(alloc.tensor_shape)
            dtype = mybir.dt.np(alloc.dtype)
            out_avals.append(jax.core.ShapedArray(shape, dtype))
            zero_outs.append(np.zeros(shape, dtype))
    n_params = len(in_names)
    all_in_names = list(in_names) + list(out_names)
    if partition_name is not None:
        all_in_names.append(partition_name)

    def _body(*args):
        operands = list(args)
        if partition_name is not None:
            operands.append(bass2jax.partition_id_tensor())
        outs = bass2jax._bass_exec_p.bind(
            *operands,
            out_avals=tuple(out_avals),
            in_names=tuple(all_in_names),
            out_names=tuple(out_names),
            lowering_input_output_aliases=(),
            sim_require_finite=True,
            sim_require_nnan=True,
            nc=nc,
        )
        return tuple(outs)

    devices = jax.devices()[:C]
    mesh = Mesh(np.asarray(devices), ("core",))
    nouts = len(out_names)
    sharded = jax.jit(
        shard_map(_body, mesh=mesh,
                  in_specs=(PartitionSpec("core"),) * (n_params + nouts),
                  out_specs=(PartitionSpec("core"),) * nouts,
                  check_rep=False),
        keep_unused=True,
    )
    concat_in = [
        np.concatenate([np.asarray(in_maps[c][name]) for c in range(C)], 0)
        for name in in_names
    ]
    concat_zeros = [
        np.zeros((C * z.shape[0], *z.shape[1:]), z.dtype) for z in zero_outs
    ]
    dev_args = [jax.device_put(a) for a in concat_in + concat_zeros]
    # warmup (compile + first exec)
    outs = sharded(*dev_args)
    jax.block_until_ready(outs)
    times = []
    for _ in range(iters):
        t0 = time.perf_counter()
        outs = sharded(*dev_args)
        jax.block_until_ready(outs)
        times.append((time.perf_counter() - t0) * 1e9)
    times.sort()
    return dict(min=int(times[0]), median=int(times[len(times) // 2]),
                all=[int(t) for t in times])


# ------------------------------------------------------------------- kernel
_last_results = {}


def kernel(x, edge_index, W0, b0, W1, b1, W2, b2):
    from concourse.bass_utils import run_bass_kernel_spmd

    x = np.asarray(x, np.float32)
    S = _build_structure(edge_index)
    nc = _build_program(S)

    node_at = S["node_at"]
    deg = S["deg"]
    bf = ml_dtypes.bfloat16

    in_maps = []
    for c in range(C):
        m = node_at[c] >= 0
        xs = np.zeros((SHARD_PAD, D), np.float32)
        xs[m] = x[node_at[c, m]]
        dcol = np.ones(SHARD_PAD, np.float32)
        dcol[m] = deg[node_at[c, m]].astype(np.float32)
        degt = np.ascontiguousarray(dcol.reshape(TILES, P).T)
        in_maps.append({
            "x_shard": xs,
            "degs": degt,
            "idxw": np.ascontiguousarray(S["idx_wrapped"][c]),
            "W0": np.asarray(W0, np.float32).astype(bf),
            "W1": np.asarray(W1, np.float32).astype(bf),
            "W2": np.asarray(W2, np.float32).astype(bf),
            "b0": np.tile(np.asarray(b0, np.float32)[None, :], (P, 1)),
            "b1": np.tile(np.asarray(b1, np.float32)[None, :], (P, 1)),
            "b2": np.tile(np.asarray(b2, np.float32)[None, :], (P, 1)),
        })

    import os
    res = run_bass_kernel_spmd(
        nc, in_maps, core_ids=list(range(C)),
        trace=bool(int(os.environ.get("KERNEL_TRACE", "0"))),
    )
    _last_results["exec_time_ns"] = res.exec_time_ns
    _last_results["results"] = res

    nbench = int(os.environ.get("KERNEL_BENCH", "0"))
    if nbench:
        _last_results["bench_ns"] = _bench_exec(nc, in_maps, nbench)

    out = np.zeros((N_NODES, OUT), np.float32)
    for c in range(C):
        m = node_at[c] >= 0
        out[node_at[c, m]] = res.results[c]["out"][m]
    return out


# revision 9
# speedup vs baseline: 1.8969x; 1.8969x over previous
"""GCN 3-layer Bass kernel for nn_ActionNetwork_20401094656134 on 8 trn2 cores.

Plan (node-parallel, dst-sharded):
- augment edges (drop self loops, add one per node); deg = in-degree
- nodes snake-dealt by degree to 8 shards; within shard, nodes get a parity
  bit (greedy discrepancy balancing of each dst's source parities) and are
  placed at matching-parity table positions sorted by (b,a) class counts
- scaled-table formulation: maintaining h~ = rsqrt(deg) * h makes each layer
      S[d] = sum_{e: dst=d} h~[src_e] + h~[d]   (self loop is a real slot)
      z    = (dis * S) @ W + b,   next h~ = dis * relu(z)
- per layer: every core holds the full bf16 h~ table in HBM (AllGather),
  gathers its ELL slots with batched SWDGE dma_gather (128B rows addressed
  as halves of 256B pairs; int16 idx = pair - 32768; even/odd source-row
  classes), segment-sums via identity-weight PE matmuls accumulating in
  PSUM, and runs the tiny per-tile tail (scale, transpose, W matmul, bias,
  relu-scale) on DVE/PE/ACT.
"""
import sys
sys.path.insert(0, "/opt/trn_rl_repo")

import numpy as np
import ml_dtypes

N_NODES = 100000
D = 64
OUT = 4
C = 8
P = 128
TILES = 98
SHARD_PAD = TILES * P          # 12544
NPS = N_NODES // C             # 12500
N_DUMMY = SHARD_PAD - NPS      # 44
IDX_BASE = 32768               # pair-unit base for int16 idx
PAD_PAIR = (6 * SHARD_PAD) // 2  # shard-6 dummy rows 75264/75265
PAD_IDX = PAD_PAIR - IDX_BASE
TABLE_ROWS = C * SHARD_PAD     # 100352
GK = 120                       # max data columns per gather group


# ---------------------------------------------------------------- host prep
def _build_structure(edge_index):
    n = N_NODES
    src = np.asarray(edge_index[0], np.int64)
    dst = np.asarray(edge_index[1], np.int64)
    keep = src != dst
    src, dst = src[keep], dst[keep]
    src = np.concatenate([src, np.arange(n, dtype=np.int64)])
    dst = np.concatenate([dst, np.arange(n, dtype=np.int64)])
    deg = np.bincount(dst, minlength=n).astype(np.int64)

    order = np.argsort(deg, kind="stable")
    shard_of = np.empty(n, np.int64)
    for i in range(0, n, 2 * C):
        blk = order[i:i + 2 * C]
        shard_of[blk[:C]] = np.arange(len(blk[:C]))
        bwd = blk[C:]
        shard_of[bwd] = np.arange(C - 1, C - 1 - len(bwd), -1)

    ord_e = np.argsort(dst, kind="stable")
    src_s = src[ord_e]
    starts = np.zeros(n + 1, np.int64)
    np.cumsum(deg, out=starts[1:])

    # greedy parity assignment balancing each dst's source parity split
    ord_s = np.argsort(src, kind="stable")
    dst_bysrc = dst[ord_s]
    odeg = np.bincount(src, minlength=n).astype(np.int64)
    sstarts = np.zeros(n + 1, np.int64)
    np.cumsum(odeg, out=sstarts[1:])

    parity = np.zeros(n, np.int8)
    imbal = np.zeros(n, np.int32)
    quota = np.zeros((C, 2), np.int64)
    quota[:, 0] = NPS // 2
    quota[:, 1] = NPS - NPS // 2
    for nd in order[::-1]:
        c = shard_of[nd]
        lo, hi = sstarts[nd], sstarts[nd + 1]
        vote = int(imbal[dst_bysrc[lo:hi]].sum())
        if quota[c, 0] == 0:
            p = 1
        elif quota[c, 1] == 0:
            p = 0
        else:
            p = 0 if vote <= 0 else 1
        parity[nd] = p
        quota[c, p] -= 1
        imbal[dst_bysrc[lo:hi]] += 1 - 2 * p

    # class counts per node
    b_cnt = np.zeros(n, np.int64)
    np.add.at(b_cnt, dst, parity[src])
    a_cnt = deg - b_cnt

    # positions: per shard, parity streams sorted by (b, a), dummies first
    pos_of = np.empty(n, np.int64)
    for c in range(C):
        nodes_c = np.where(shard_of == c)[0]
        pc = parity[nodes_c]
        ev = nodes_c[pc == 0]
        od = nodes_c[pc == 1]
        ev = ev[np.lexsort((a_cnt[ev], b_cnt[ev]))]
        od = od[np.lexsort((a_cnt[od], b_cnt[od]))]
        pos_of[ev] = 2 * (SHARD_PAD // 2 - len(ev)) + 2 * np.arange(len(ev))
        pos_of[od] = 2 * (SHARD_PAD // 2 - len(od)) + 2 * np.arange(len(od)) + 1

    table_row = shard_of * SHARD_PAD + pos_of
    src_rows = table_row[src_s]
    par = src_rows & 1
    b_cnt = np.zeros(n, np.int64)
    np.add.at(b_cnt, dst[ord_e], par)
    a_cnt = deg - b_cnt

    node_at = np.full((C, SHARD_PAD), -1, np.int64)
    for c in range(C):
        nodes_c = np.where(shard_of == c)[0]
        node_at[c, pos_of[nodes_c]] = nodes_c

    a_pad = np.zeros((C, SHARD_PAD), np.int64)
    b_pad = np.zeros((C, SHARD_PAD), np.int64)
    m = node_at >= 0
    a_pad[m] = a_cnt[node_at[m]]
    b_pad[m] = b_cnt[node_at[m]]
    A_t = np.maximum(a_pad.reshape(C, TILES, P).max(axis=(0, 2)), 1)
    B_t = np.maximum(b_pad.reshape(C, TILES, P).max(axis=(0, 2)), 1)

    # groups of tiles
    groups = []
    cur, cur_cols = [], 0
    for t in range(TILES):
        cols = int(A_t[t] + B_t[t])
        if cur and cur_cols + cols > GK:
            groups.append(cur)
            cur, cur_cols = [], 0
        cur.append(t)
        cur_cols += cols
    if cur:
        groups.append(cur)

    ge_cols = [int(sum(A_t[t] for t in g)) + 1 for g in groups]  # +1 pad col
    go_cols = [int(sum(B_t[t] for t in g)) + 1 for g in groups]
    tot_cols = sum(ge_cols) + sum(go_cols)

    pair_idx = (src_rows >> 1) - IDX_BASE

    # per-core column-major slot grid [tot_cols, P], then wrap to int16 SBUF
    idx_wrapped = np.empty((C, P, 8 * tot_cols), np.int16)
    for c in range(C):
        cols_all = np.full((tot_cols, P), PAD_IDX, np.int64)
        col0 = 0
        for gi, g in enumerate(groups):
            for t in g:
                for p in range(P):
                    nd = node_at[c, t * P + p]
                    if nd < 0:
                        continue
                    s0, s1 = starts[nd], starts[nd + 1]
                    pe = pair_idx[s0:s1][par[s0:s1] == 0]
                    cols_all[col0:col0 + len(pe), p] = pe
                col0 += int(A_t[t])
            col0 += 1
            for t in g:
                for p in range(P):
                    nd = node_at[c, t * P + p]
                    if nd < 0:
                        continue
                    s0, s1 = starts[nd], starts[nd + 1]
                    po = pair_idx[s0:s1][par[s0:s1] == 1]
                    cols_all[col0:col0 + len(po), p] = po
                col0 += int(B_t[t])
            col0 += 1
        assert col0 == tot_cols
        L = cols_all.reshape(-1).astype(np.int16)
        Mw = L.reshape(-1, 16).T
        idx_wrapped[c] = np.tile(Mw, (8, 1))

    return dict(
        deg=deg, node_at=node_at, groups=groups, A_t=A_t, B_t=B_t,
        ge_cols=ge_cols, go_cols=go_cols, tot_cols=tot_cols,
        idx_wrapped=idx_wrapped,
    )


# ------------------------------------------------- patched batched dma_gather
def _dma_gather_128(gpsimd, mybir, ap_utils, out_ap, in_ap, idxs_ap,
                    num_idxs, elem_size, elem_step, queue_num):
    """dma_gather with 128-byte elements (relaxes bass's %256 assert; the Q7
    non-transpose path has no such requirement — only the 256B *stride*
    granularity is ISA-level, which the pair-view satisfies)."""
    from concourse.bass import MemorySpace, exact_div, round_up_to_multiple
    self = gpsimd
    assert idxs_ap.dtype == mybir.dt.int16
    assert in_ap.dtype == out_ap.dtype
    elem_size_bytes = elem_size * mybir.dt.size(in_ap.dtype)
    assert elem_size_bytes % 128 == 0
    assert in_ap.space == MemorySpace.DRAM
    assert ap_utils.ap_is_contiguous(out_ap.ap[1:])
    assert ap_utils.ap_is_contiguous(idxs_ap.ap[1:])
    assert num_idxs % 128 == 0
    assert in_ap.ap[-1][1] == out_ap.ap[-1][1] == elem_size
    assert out_ap.ap[0][1] * out_ap.ap[1][1] == round_up_to_multiple(num_idxs, 128)
    assert in_ap.ap[0][0] == elem_step
    stride_bytes = elem_step * mybir.dt.size(in_ap.dtype)
    stride_bytes_256 = exact_div(stride_bytes, 256)
    assert stride_bytes_256 < 256
    self._assert_queue_num(queue_num)

    _in_ap = self.lower_ap_dma(in_ap, for_custom_bir_dma=True)
    _idxs_ap = self.lower_ap(idxs_ap)
    _out_ap = self.lower_ap(out_ap)
    return self.add_instruction(
        mybir.InstDMAGatherAnt(
            name=self.bass.get_next_instruction_name(),
            ins=[*_in_ap, _idxs_ap,
                 self.lower_val_access(self.to_reg(num_idxs))],
            outs=[_out_ap],
            transpose=False,
            num_idxs=num_idxs,
            elem_size=elem_size,
            stride_bytes_256=stride_bytes_256,
            gen_mode=0,
            single_packet=False,
            queue_num=queue_num,
            sbuf_tokens_per_rank=0,
            sbuf_free_dim_per_rank=0,
            sbuf_free_dim_pad_per_rank=0,
            sbuf_byte_offset=0,
        )
    )


# ------------------------------------------------------------ device program
def _build_program(S, single_core=False):
    import concourse.bass as bass
    import concourse.bacc as bacc
    import concourse.tile as tile
    import concourse.mybir as mybir
    import concourse.ap_utils as ap_utils
    from concourse.masks import make_identity

    bf16 = mybir.dt.bfloat16
    f32 = mybir.dt.float32
    groups, A_t, B_t = S["groups"], S["A_t"], S["B_t"]
    ge_cols, go_cols, tot_cols = S["ge_cols"], S["go_cols"], S["tot_cols"]
    max_gcols = max(ge + go for ge, go in zip(ge_cols, go_cols))

    nc = bacc.Bacc("TRN2", target_bir_lowering=False, debug=False,
                   num_devices=1 if single_core else C, num_swdge_queues=4,
                   dynamic_dma_scratch_size=65536)

    in_x = nc.dram_tensor("x_shard", (SHARD_PAD, D), f32, kind="ExternalInput").ap()
    in_deg = nc.dram_tensor("degs", (P, TILES), f32, kind="ExternalInput").ap()
    in_idx = nc.dram_tensor("idxw", (P, 8 * tot_cols), mybir.dt.int16,
                            kind="ExternalInput").ap()
    in_W0 = nc.dram_tensor("W0", (D, D), bf16, kind="ExternalInput").ap()
    in_W1 = nc.dram_tensor("W1", (D, D), bf16, kind="ExternalInput").ap()
    in_W2 = nc.dram_tensor("W2", (D, OUT), bf16, kind="ExternalInput").ap()
    in_b0 = nc.dram_tensor("b0", (P, D), f32, kind="ExternalInput").ap()
    in_b1 = nc.dram_tensor("b1", (P, D), f32, kind="ExternalInput").ap()
    in_b2 = nc.dram_tensor("b2", (P, OUT), f32, kind="ExternalInput").ap()
    out_t = nc.dram_tensor("out", (SHARD_PAD, OUT), f32, kind="ExternalOutput").ap()

    with tile.TileContext(nc) as tc:
        with tc.tile_pool(name="const", bufs=1) as constp, \
             tc.tile_pool(name="dram", bufs=1, space="DRAM") as dram, \
             tc.tile_pool(name="xin", bufs=3) as xin, \
             tc.tile_pool(name="stgp", bufs=3) as stgp, \
             tc.tile_pool(name="work", bufs=3) as work, \
             tc.tile_pool(name="accp", bufs=3, space="PSUM") as accp, \
             tc.tile_pool(name="tpsp", bufs=2, space="PSUM") as tpsp, \
             tc.tile_pool(name="zpsp", bufs=2, space="PSUM") as zpsp:

            ident = constp.tile([P, P], bf16)
            make_identity(nc, ident[:])
            W0_sb = constp.tile([D, D], bf16, tag="W0")
            W1_sb = constp.tile([D, D], bf16, tag="W1")
            W2_sb = constp.tile([D, OUT], bf16, tag="W2")
            W_sb = [W0_sb, W1_sb, W2_sb]
            nc.sync.dma_start(W_sb[0][:], in_W0[:])
            nc.sync.dma_start(W_sb[1][:], in_W1[:])
            nc.sync.dma_start(W_sb[2][:], in_W2[:])
            b0_sb = constp.tile([P, D], f32, tag="b0")
            b1_sb = constp.tile([P, D], f32, tag="b1")
            b2_sb = constp.tile([P, OUT], f32, tag="b2")
            b_sb = [b0_sb, b1_sb, b2_sb]
            nc.sync.dma_start(b_sb[0][:], in_b0[:])
            nc.sync.dma_start(b_sb[1][:], in_b1[:])
            nc.sync.dma_start(b_sb[2][:], in_b2[:])

            idx_sb = constp.tile([P, 8 * tot_cols], mybir.dt.int16)
            nc.sync.dma_start(idx_sb[:], in_idx[:])

            deg_sb = constp.tile([P, TILES], f32)
            nc.sync.dma_start(deg_sb[:], in_deg[:])
            rcp = constp.tile([P, TILES], f32)
            nc.vector.reciprocal(rcp[:], deg_sb[:])
            dis = constp.tile([P, TILES], f32)
            nc.scalar.sqrt(dis[:], rcp[:])

            zpad = constp.tile([P, D], bf16)
            nc.gpsimd.memset(zpad[:], 0.0)

            tblA = dram.tile([TABLE_ROWS, D], bf16)
            tblB = dram.tile([TABLE_ROWS, D], bf16)
            agin = dram.tile([SHARD_PAD, D], bf16)

            def do_allgather(dst):
                if single_core:
                    for cc in range(C):
                        nc.sync.dma_start(
                            dst[cc * SHARD_PAD:(cc + 1) * SHARD_PAD, :], agin[:])
                else:
                    nc.gpsimd.collective_compute(
                        "AllGather", mybir.AluOpType.bypass,
                        replica_groups=[list(range(C))],
                        ins=[agin[:].opt()], outs=[dst[:].opt()],
                    )

            # ---- layer-0 table: h0~ = dis * x (batched 4 tiles per DMA) ----
            for t0 in range(0, TILES, 4):
                nb = min(4, TILES - t0)
                xt = xin.tile([P, 4 * D], f32, tag="xt")
                nc.sync.dma_start(
                    xt[:, :nb * D].rearrange("p (j d) -> p j d", j=nb),
                    in_x[t0 * P:(t0 + nb) * P, :].rearrange(
                        "(j p) d -> p j d", p=P))
                h0 = xin.tile([P, 4 * D], bf16, tag="h0")
                for j in range(nb):
                    nc.vector.tensor_scalar_mul(
                        h0[:, j * D:(j + 1) * D], xt[:, j * D:(j + 1) * D],
                        dis[:, t0 + j:t0 + j + 1])
                nc.sync.dma_start(
                    agin[t0 * P:(t0 + nb) * P, :].rearrange(
                        "(j p) d -> p j d", p=P),
                    h0[:, :nb * D].rearrange("p (j d) -> p j d", j=nb))
            do_allgather(tblA[:])

            qrr = 0
            for layer in range(3):
                table = tblA if layer % 2 == 0 else tblB
                tblP = table[:].rearrange("(r two) d -> r (two d)", two=2)
                Wl = W_sb[layer]
                bl = b_sb[layer]
                DO = D if layer < 2 else OUT
                wcol0 = 0
                for gi, g in enumerate(groups):
                    Acols, Bcols = ge_cols[gi], go_cols[gi]
                    cols_g = Acols + Bcols
                    stg = stgp.tile([P, max_gcols * D], bf16, tag="stg")
                    stg3 = stg[:].rearrange("p (k d) -> p k d", k=max_gcols)
                    _dma_gather_128(
                        nc.gpsimd, mybir, ap_utils,
                        stg3[:, 0:Acols, :], tblP[IDX_BASE:, 0:D],
                        idx_sb[:, 8 * wcol0:8 * (wcol0 + Acols)],
                        num_idxs=P * Acols, elem_size=D, elem_step=2 * D,
                        queue_num=qrr % 4,
                    )
                    _dma_gather_128(
                        nc.gpsimd, mybir, ap_utils,
                        stg3[:, Acols:cols_g, :], tblP[IDX_BASE:, D:2 * D],
                        idx_sb[:, 8 * (wcol0 + Acols):8 * (wcol0 + cols_g)],
                        num_idxs=P * Bcols, elem_size=D, elem_step=2 * D,
                        queue_num=(qrr + 1) % 4,
                    )
                    qrr += 2
                    wcol0 += cols_g

                    aoff = 0
                    boff = Acols
                    tiles_g = list(g)
                    for bt0 in range(0, len(tiles_g), 4):
                        batch = tiles_g[bt0:bt0 + 4]
                        nb = len(batch)
                        wide = work.tile([P, 4 * DO], bf16 if layer < 2 else f32,
                                         tag="wide")
                        for j, t in enumerate(batch):
                            ka, kb = int(A_t[t]), int(B_t[t])
                            acc = accp.tile([P, D], f32, space="PSUM", tag="acc")
                            slot_cols = ([aoff + k for k in range(ka)] +
                                         [boff + k for k in range(kb)])
                            for jj, col in enumerate(slot_cols):
                                nc.tensor.matmul(
                                    out=acc[:], lhsT=ident[:],
                                    rhs=stg[:, col * D:(col + 1) * D],
                                    start=(jj == 0),
                                    stop=(jj == len(slot_cols) - 1),
                                )
                            aoff += ka
                            boff += kb

                            ssc = work.tile([P, D], bf16, tag="ssc")
                            nc.vector.tensor_scalar_mul(ssc[:], acc[:],
                                                        dis[:, t:t + 1])
                            tps = tpsp.tile([D, P], bf16, space="PSUM", tag="tps")
                            nc.tensor.transpose(out=tps[:], in_=ssc[:],
                                                identity=ident[:])
                            stt = work.tile([D, P], bf16, tag="stt")
                            nc.vector.tensor_copy(stt[:], tps[:])
                            zps = zpsp.tile([P, DO], f32, space="PSUM", tag="zps")
                            nc.tensor.matmul(out=zps[:], lhsT=stt[:], rhs=Wl[:],
                                             start=True, stop=True)
                            if layer < 2:
                                zf = work.tile([P, D], f32, tag="zf")
                                nc.vector.tensor_tensor(out=zf[:], in0=zps[:],
                                                        in1=bl[:],
                                                        op=mybir.AluOpType.add)
                                nc.scalar.activation(
                                    wide[:, j * D:(j + 1) * D], zf[:],
                                    mybir.ActivationFunctionType.Relu,
                                    scale=dis[:, t:t + 1])
                            else:
                                nc.vector.tensor_tensor(
                                    out=wide[:, j * OUT:(j + 1) * OUT],
                                    in0=zps[:], in1=bl[:],
                                    op=mybir.AluOpType.add)
                        t0 = batch[0]
                        if layer < 2:
                            nc.sync.dma_start(
                                agin[t0 * P:(t0 + nb) * P, :].rearrange(
                                    "(j p) d -> p j d", p=P),
                                wide[:, :nb * D].rearrange(
                                    "p (j d) -> p j d", j=nb))
                        else:
                            nc.sync.dma_start(
                                out_t[t0 * P:(t0 + nb) * P, :].rearrange(
                                    "(j p) d -> p j d", p=P),
                                wide[:, :nb * OUT].rearrange(
                                    "p (j d) -> p j d", j=nb))

                if layer < 2:
                    # dummy rows must stay exactly zero in the table
                    nc.sync.dma_start(agin[0:N_DUMMY, :], zpad[0:N_DUMMY, :])
                    nxt = tblB if layer % 2 == 0 else tblA
                    do_allgather(nxt[:])

    nc.compile()
    return nc


def _bench_exec(nc, in_maps, iters):
    """Steady-state timing of the NEFF via repeated PJRT executions (no
    donation, device-resident inputs). Returns min per-iteration ns."""
    import time
    import jax
    import numpy as np
    from jax.sharding import Mesh, PartitionSpec
    from jax.experimental.shard_map import shard_map
    import concourse.mybir as mybir
    from concourse import bass2jax

    bass2jax.install_neuronx_cc_hook()
    partition_name = (nc.partition_id_tensor.name
                      if nc.partition_id_tensor else None)
    in_names, out_names, out_avals, zero_outs = [], [], [], []
    for alloc in nc.m.functions[0].allocations:
        if not isinstance(alloc, mybir.MemoryLocationSet):
            continue
        name = alloc.memorylocations[0].name
        if alloc.kind == "ExternalInput":
            if name != partition_name:
                in_names.append(name)
        elif alloc.kind == "ExternalOutput":
            out_names.append(name)
            shape = tuple(alloc.tensor_shape)
            dtype = mybir.dt.np(alloc.dtype)
            out_avals.append(jax.core.ShapedArray(shape, dtype))
            zero_outs.append(np.zeros(shape, dtype))
    n_params = len(in_names)
    all_in_names = list(in_names) + list(out_names)
    if partition_name is not None:
        all_in_names.append(partition_name)

    def _body(*args):
        operands = list(args)
        if partition_name is not None:
            operands.append(bass2jax.partition_id_tensor())
        outs = bass2jax._bass_exec_p.bind(
            *operands,
            out_avals=tuple(out_avals),
            in_names=tuple(all_in_names),
            out_names=tuple(out_names),
            lowering_input_output_aliases=(),
            sim_require_finite=True,
            sim_require_nnan=True,
            nc=nc,
        )
        return tuple(outs)

    devices = jax.devices()[:C]
    mesh = Mesh(np.asarray(devices), ("core",))
    nouts = len(out_names)
    sharded = jax.jit(
        shard_map(_body, mesh=mesh,
                  in_specs=(PartitionSpec("core"),) * (n_params + nouts),
                  out_specs=(PartitionSpec("core"),) * nouts,
                  check_rep=False),
        keep_unused=True,
    )
    concat_in = [
        np.concatenate([np.asarray(in_maps[c][name]) for c in range(C)], 0)
        for name in in_names
    ]
    concat_zeros = [
        np.zeros((C * z.shape[0], *z.shape[1:]), z.dtype) for z in zero_outs
    ]
    dev_args = [jax.device_put(a) for a in concat_in + concat_zeros]
    # warmup (compile + first exec)
    outs = sharded(*dev_args)
    jax.block_until_ready(outs)
    times = []
    for _ in range(iters):
        t0 = time.perf_counter()
        outs = sharded(*dev_args)
        jax.block_until_ready(outs)
        times.append((time.perf_counter() - t0) * 1e9)
    times.sort()
    return dict(min=int(times[0]), median=int(times[len(times) // 2]),
                all=[int(t) for t in times])


# ------------------------------------------------------------------- kernel
_last_results = {}


def kernel(x, edge_index, W0, b0, W1, b1, W2, b2):
    from concourse.bass_utils import run_bass_kernel_spmd

    x = np.asarray(x, np.float32)
    S = _build_structure(edge_index)
    nc = _build_program(S)

    node_at = S["node_at"]
    deg = S["deg"]
    bf = ml_dtypes.bfloat16

    in_maps = []
    for c in range(C):
        m = node_at[c] >= 0
        xs = np.zeros((SHARD_PAD, D), np.float32)
        xs[m] = x[node_at[c, m]]
        dcol = np.ones(SHARD_PAD, np.float32)
        dcol[m] = deg[node_at[c, m]].astype(np.float32)
        degt = np.ascontiguousarray(dcol.reshape(TILES, P).T)
        in_maps.append({
            "x_shard": xs,
            "degs": degt,
            "idxw": np.ascontiguousarray(S["idx_wrapped"][c]),
            "W0": np.asarray(W0, np.float32).astype(bf),
            "W1": np.asarray(W1, np.float32).astype(bf),
            "W2": np.asarray(W2, np.float32).astype(bf),
            "b0": np.tile(np.asarray(b0, np.float32)[None, :], (P, 1)),
            "b1": np.tile(np.asarray(b1, np.float32)[None, :], (P, 1)),
            "b2": np.tile(np.asarray(b2, np.float32)[None, :], (P, 1)),
        })

    import os
    res = run_bass_kernel_spmd(
        nc, in_maps, core_ids=list(range(C)),
        trace=bool(int(os.environ.get("KERNEL_TRACE", "0"))),
    )
    _last_results["exec_time_ns"] = res.exec_time_ns
    _last_results["results"] = res

    nbench = int(os.environ.get("KERNEL_BENCH", "0"))
    if nbench:
        _last_results["bench_ns"] = _bench_exec(nc, in_maps, nbench)

    out = np.zeros((N_NODES, OUT), np.float32)
    for c in range(C):
        m = node_at[c] >= 0
        out[node_at[c, m]] = res.results[c]["out"][m]
    return out
